# revision 56
# baseline (speedup 1.0000x reference)
"""Trainium2 Bass kernel for DihedralAngleEncoder.

Computes phi/psi/omega backbone dihedral sin/cos features and projects
them 6->64 with a linear layer, for coords [64, 4096, 4, 3].

Math notes (vs. the jax reference):
  - cos(sign*arccos(c)) == c, and sin(sign*arccos(c)) == sign*sqrt(1-c^2),
    so arccos/sin/cos are never evaluated.
  - sign(n1_normalized . v3) == sign(n1 . v3) (norms are positive).
  - cos = Q / (sqrt(Pa + tiny) * sqrt(Pb + tiny)) instead of the
    reference's Q / ((ra+eps)(rb+eps)); the difference is O(1e-8).
  - Boundary duplications (phi i==0, psi i==L-1) are baked into the
    host-side input repack; omega at i==L-1 degenerates to exactly
    sin=0, cos=1 which is patched in as constants.

Layout ("c-interleaved"): position pos = 8*g + c with g = ghi*256 + glo.
SBUF partition = c*16 + ghi, free col = glo. The host pre-packs the
input as 18 planes (6 shifted atom copies x xyz) in this layout, so the
whole elementwise pipeline runs on aligned 128-partition operands with
no on-chip shifts. Features are written in bf16; the feature->stationary
gather is 7 per-j-plane DMAs per unit (partition-crossing AP strides
must stay in the leading dims on both sides), and the projection PSUM
comes out position-major so output DMAs get 2KB-contiguous HBM runs at
full DMA rate.

Projection: lhsT = T56 [56 = (6 feats + ones) x 8 c-blocks, g-cols]
against a block-diagonal [56, 512] bf16 weight (1 PE cycle/row, 4x
faster streaming than fp32; fp32r is rejected by the BIR verifier /
broken on HW for this pattern).

Pipeline: 5 elementwise chunks of decreasing size feed 3
gather/matmul/output units so the output-DMA stream starts early and
the tail is short. All HWDGE DMAs issue from queues whose program order
matches data-readiness order (input loads + gathers on SP, outputs on
Act after their copies, boundary patches on Pool/SWDGE) because a DMA
whose semaphore wait is not yet satisfied blocks everything behind it
on that sequencer.

Sharding: pure data parallel over the batch dim, 8 batch rows per core.
"""

import sys
from contextlib import ExitStack

import numpy as np

if "/opt/trn_rl_repo" not in sys.path:
    sys.path.insert(0, "/opt/trn_rl_repo")

B, L = 64, 4096
NCORES = 8
PB = B // NCORES            # batch rows per core
NPOS = PB * L               # 32768 positions per core
G = NPOS // 8               # 4096 groups of 8 consecutive positions
GHI = 16                    # partition = c*16 + ghi
GLO = 256                   # free cols per partition
NPL = 18                    # input planes (6 atoms x xyz)
NCH = 2                     # glo chunks (pipeline stages)
W = GLO // NCH              # 128 cols per chunk
TINY = 1e-12

_CACHE = {}


def _build_module():
    import concourse.bass as bass
    import concourse.bacc as bacc
    import concourse.tile as tile
    from concourse import mybir

    f32 = mybir.dt.float32
    f32r = mybir.dt.float32r
    bf16 = mybir.dt.bfloat16
    Alu = mybir.AluOpType
    Act = mybir.ActivationFunctionType

    nc = bacc.Bacc(trn_type="TRN2")
    tiny_t = nc.alloc_sbuf_tensor("const-tiny", [128, 1], f32)
    nc.gpsimd.memset(tiny_t.ap(), TINY)
    nc.const_aps.aps[(f32, TINY)] = tiny_t.ap()
    xh = nc.dram_tensor("xh", [128, NPL * GLO], f32, kind="ExternalInput")
    cz = nc.dram_tensor("cz", [128, 2], bf16, kind="ExternalInput")
    w8 = nc.dram_tensor("w8", [56, 512], bf16, kind="ExternalInput")
    out = nc.dram_tensor("out", [NPOS, 64], f32, kind="ExternalOutput")

    with tile.TileContext(nc) as tc, ExitStack() as ctx:
        singles = ctx.enter_context(tc.tile_pool(name="singles", bufs=1))
        work = ctx.enter_context(tc.tile_pool(name="work", bufs=1))
        psum = ctx.enter_context(tc.tile_pool(name="psum", bufs=2, space="PSUM"))
        outp = ctx.enter_context(tc.tile_pool(name="outp", bufs=4))

        W8sb = singles.tile([56, 512], bf16)

        xh3 = xh[:, :].rearrange("p (pl g) -> p pl g", pl=NPL)

        # EW chunks (gl0, Wc) feed gather/matmul/output units. Units:
        #   U0 = ch0 (64-wide, starts the output stream early)
        #   U1 = ch1+ch2 (128-wide, consolidated per-psum-tile outputs)
        #   U2 = ch3+ch4 (64-wide, short output tail)
        CHS = [(0, 64), (64, 64), (128, 64), (192, 32), (224, 32)]
        UNITS = {0: (0, 64, [0]), 1: (64, 128, [1, 2]), 2: (192, 64, [3, 4])}
        CH2UNIT = {0: 0, 1: 1, 2: 1, 3: 2, 4: 2}
        XL = {}
        for half in (0, 1):
            t = work.tile([128, NPL * 128], f32, tag=f"XL{half}", name=f"XL{half}")
            nc.sync.dma_start(
                out=t, in_=xh3[:, :, 128 * half : 128 * (half + 1)]
            )
            XL[half] = t
        nc.sync.dma_start(out=W8sb, in_=w8[:, :])
        FU = {}
        for ci, (gl0, Wc) in enumerate(CHS):
            half = gl0 // 128
            off = gl0 - 128 * half
            W = Wc
            XC3 = XL[half].rearrange("p (pl t) -> p pl t", pl=NPL)[
                :, :, off : off + W
            ]

            ui = CH2UNIT[ci]
            ugl0, Wu, uchunks = UNITS[ui]
            if ui not in FU:
                FU[ui] = work.tile([128, 7 * Wu], bf16, tag=f"F{ui}", name=f"F{ui}")
                FJ = FU[ui].rearrange("p (j t) -> p j t", j=7)
                nc.vector.memset(FJ[:, 6:7, :], 1.0)
            F = FU[ui]
            foff = gl0 - ugl0  # column offset of this chunk inside F planes
            F7 = F.rearrange("p (j t) -> p j t", j=7)

            # ---- stage 1: 5 difference vectors (15 planes) ----
            V = work.tile([128, 15 * W], f32, tag=f"V{ci}")
            V3 = V.rearrange("p (pl t) -> p pl t", pl=15)
            V4 = V.rearrange("p (v k t) -> p v k t", v=5, k=3)
            nc.vector.tensor_sub(V3, XC3[:, 3:18, :], XC3[:, 0:15, :])

            # ---- stage 2: cross products A=v1xv2 B=v2xv3 C=v3xv4 M=v4xv5 ----
            T1 = work.tile([128, 12 * W], f32, tag=f"T1{ci}")
            T2 = work.tile([128, 12 * W], f32, tag=f"T2{ci}")
            T14 = T1.rearrange("p (x k t) -> p x k t", x=4, k=3)
            T24 = T2.rearrange("p (x k t) -> p x k t", x=4, k=3)
            for k in range(3):
                p1, p2 = (k + 1) % 3, (k + 2) % 3
                eng1 = nc.vector if k == 1 else nc.gpsimd
                eng1.tensor_mul(T14[:, :, k, :], V4[:, 0:4, p1, :], V4[:, 1:5, p2, :])
                eng2 = nc.vector if k == 2 else nc.gpsimd
                eng2.tensor_mul(T24[:, :, k, :], V4[:, 0:4, p2, :], V4[:, 1:5, p1, :])
            XP = T1
            nc.vector.tensor_sub(XP, T1, T2)

            # ---- stage 3: 10 dot products (30 product planes) ----
            PR = work.tile([128, 30 * W], f32, tag=f"PR{ci}")
            nc.vector.tensor_mul(
                PR[:, 0 : 9 * W], XP[:, 0 : 9 * W], XP[:, 3 * W : 12 * W]
            )
            nc.scalar.activation(
                PR[:, 9 * W : 21 * W], XP[:, 0 : 12 * W], Act.Square
            )
            nc.gpsimd.tensor_mul(
                PR[:, 21 * W : 30 * W], XP[:, 0 : 9 * W], V[:, 6 * W : 15 * W]
            )
            PR4 = PR.rearrange("p (d k t) -> p d k t", d=10, k=3)
            DOT = work.tile([128, 10 * W], f32, tag=f"DOT{ci}")
            DOT2 = DOT.rearrange("p (d t) -> p d t", d=10)
            nc.vector.tensor_add(DOT2, PR4[:, :, 0, :], PR4[:, :, 1, :])
            nc.gpsimd.tensor_add(DOT2, DOT2, PR4[:, :, 2, :])

            # ---- stage 4: angles -> sin/cos features ----
            Q = DOT[:, 0 : 3 * W]
            Pn = DOT[:, 3 * W : 7 * W]
            S = DOT[:, 7 * W : 10 * W]
            SCR = work.tile([128, 12 * W], f32, tag=f"SCR{ci}")
            R = SCR[:, 0 : 4 * W]
            DEN = SCR[:, 4 * W : 7 * W]
            INV = SCR[:, 7 * W : 10 * W]
            CRAW = SCR[:, 4 * W : 7 * W]
            nc.scalar.activation(R, Pn, Act.Sqrt, bias=TINY)
            nc.vector.tensor_mul(DEN, R[:, 0 : 3 * W], R[:, W : 4 * W])
            nc.vector.reciprocal(INV, DEN)

            Fsin = F7[:, 0:3, foff : foff + W]
            Fcos = F7[:, 3:6, foff : foff + W]
            nc.vector.tensor_mul(CRAW, Q, INV)
            CR3 = SCR.rearrange("p (d t) -> p d t", d=12)[:, 4:7, :]
            nc.vector.tensor_scalar(
                Fcos, CR3, -1.0, 1.0, op0=Alu.max, op1=Alu.min
            )
            SIN0 = SCR[:, 0 : 3 * W]
            SG = SCR[:, 7 * W : 10 * W]
            S2f = SCR[:, 3 * W : 6 * W]
            S3 = SCR.rearrange("p (d t) -> p d t", d=12)
            nc.scalar.activation(S3[:, 3:6, :], Fcos, Act.Square)
            nc.scalar.activation(SIN0, S2f, Act.Sqrt, scale=-1.0, bias=1.0)
            nc.scalar.activation(SG, S, Act.Sign)
            nc.vector.tensor_mul(Fsin, S3[:, 7:10, :], S3[:, 0:3, :])

            if ci == len(CHS) - 1:
                # omega at the last residue of each batch row (partition
                # c=7, ghi odd; glo = 255 -> F col offset foff + W - 1).
                e = foff + W - 1
                nc.gpsimd.dma_start(
                    out=F7[113:128:2, 2:3, e : e + 1], in_=cz[113:128:2, 0:1]
                )
                nc.gpsimd.dma_start(
                    out=F7[113:128:2, 5:6, e : e + 1], in_=cz[113:128:2, 1:2]
                )


        for ui in range(3):
            ugl0, Wu, uchunks = UNITS[ui]
            F = FU[ui]

            # ---- gather F -> stationary T56 (7 per-j DMAs) ----
            # T56[j*8+c, ghi*Wu + glo] = F[c*16+ghi, j*Wu + glo]. Partition
            # strides must stay in the leading AP dims on both sides, so one
            # j-plane per DMA: src is a plain column slice of F, dst folds
            # 128 partitions into 8 rows of 16 column blocks.
            T56 = work.tile([56, 16 * Wu], bf16, tag=f"T56{ui}")
            fp = F.ap[0][0]
            tp = T56.ap[0][0]
            assert tp == 16 * Wu, (tp, Wu)
            for j in range(7):
                gsrc = bass.AP(
                    tensor=F.tensor,
                    offset=F.offset + j * Wu,
                    ap=[[fp, 128], [1, Wu]],
                )
                gdst = bass.AP(
                    tensor=T56.tensor,
                    offset=T56.offset + 8 * j * tp,
                    ap=[[tp, 8], [Wu, 16], [1, Wu]],
                )
                (nc.sync if j < 5 else nc.gpsimd).dma_start(out=gdst, in_=gsrc)

            # ---- projection ----
            # 128-wide units: 4 wide matmuls [56,128] per psum tile, one
            # consolidated output DMA per tile (PSUM partition = g-column,
            # 2KB-contiguous HBM runs).
            # 64-wide units: 16 narrow matmuls [56,64], two per psum bank row
            # (partition halves); one output DMA per partition half.
            tail = ui == 2
            if Wu == 128:
                for t in range(4):
                    ps = psum.tile([128, 2048], f32, tag="ps")
                    ob = outp.tile([128, 2048], f32, tag="ob")
                    for i in range(4):
                        m = 4 * t + i
                        nc.tensor.matmul(
                            ps[:, 512 * i : 512 * (i + 1)],
                            lhsT=T56[:, 128 * m : 128 * (m + 1)],
                            rhs=W8sb[:, :],
                            start=True,
                            stop=True,
                        )
                    nc.scalar.copy(ob[:, 0:1024], ps[:, 0:1024])
                    nc.vector.tensor_copy(ob[:, 1024:2048], ps[:, 1024:2048])
                    obp = ob.ap[0][0]
                    dsth = bass.AP(
                        tensor=out,
                        offset=131072 * 4 * t + 512 * ugl0,
                        ap=[[512, 128], [131072, 4], [1, 512]],
                    )
                    srch = bass.AP(
                        tensor=ob.tensor,
                        offset=ob.offset,
                        ap=[[obp, 128], [512, 4], [1, 512]],
                    )
                    nc.scalar.dma_start(out=dsth, in_=srch)
            else:
                assert Wu == 64
                for t in range(2):
                    ps = psum.tile([128, 2048], f32, tag="ps")
                    ob = outp.tile([128, 2048], f32, tag="ob")
                    for i in range(8):
                        ghi = 8 * t + i
                        r, k = i % 2, i // 2
                        nc.tensor.matmul(
                            ps[64 * r : 64 * r + 64, 512 * k : 512 * (k + 1)],
                            lhsT=T56[:, 64 * ghi : 64 * (ghi + 1)],
                            rhs=W8sb[:, :],
                            start=True,
                            stop=True,
                        )
                    nc.scalar.copy(ob[:, 0:1024], ps[:, 0:1024])
                    nc.vector.tensor_copy(ob[:, 1024:2048], ps[:, 1024:2048])
                    obp = ob.ap[0][0]
                    for r in range(2):
                        dsth = bass.AP(
                            tensor=out,
                            offset=131072 * (8 * t + r) + 512 * ugl0,
                            ap=[[512, 64], [262144, 4], [1, 512]],
                        )
                        srch = bass.AP(
                            tensor=ob.tensor,
                            offset=ob.offset + 64 * r * obp,
                            ap=[[obp, 64], [512, 4], [1, 512]],
                        )
                        nc.scalar.dma_start(out=dsth, in_=srch)

    nc.compile()
    return nc


def _get_nc():
    if "nc" not in _CACHE:
        _CACHE["nc"] = _build_module()
    return _CACHE["nc"]


def _run(in_maps, trace=False, **kw):
    from concourse import bass_utils

    nc = _get_nc()
    return bass_utils.run_bass_kernel_spmd(
        nc, in_maps, core_ids=list(range(NCORES)), trace=trace, **kw
    )


def _make_in_maps(backbone_coords, W, b):
    coords = np.asarray(backbone_coords, dtype=np.float32)
    Wm = np.asarray(W, dtype=np.float32)
    b = np.asarray(b, dtype=np.float32)
    # block-diagonal weights: row j*8+c = feature j of block c; row 48+c = bias
    import ml_dtypes
    w8 = np.zeros((56, 512), dtype=np.float32)
    for c in range(8):
        for j in range(6):
            w8[j * 8 + c, 64 * c : 64 * (c + 1)] = Wm.T[j]  # [64]
        w8[48 + c, 64 * c : 64 * (c + 1)] = b
    in_maps = []
    for i in range(NCORES):
        cs = coords[PB * i : PB * (i + 1)]  # [8, 4096, 4, 3]
        N, CA, C = cs[:, :, 0, :], cs[:, :, 1, :], cs[:, :, 2, :]
        C_prev = np.concatenate([C[:, :1], C[:, :-1]], axis=1)
        N_next = np.concatenate([N[:, 1:], N[:, -1:]], axis=1)
        CA_next = np.concatenate([CA[:, 1:], CA[:, -1:]], axis=1)
        planes = np.stack([C_prev, N, CA, C, N_next, CA_next], axis=0)
        # [6 atoms, PB, L, 3] -> [18 planes, NPOS]; pos = 8*g + c
        planes = planes.transpose(0, 3, 1, 2).reshape(NPL, NPOS)
        P2 = planes.reshape(NPL, GHI, GLO, 8)  # [pl, ghi, glo, c]
        xh_arr = P2.transpose(3, 1, 0, 2).reshape(128, NPL * GLO)
        czv = np.tile(
            np.array([[0.0, 1.0]], dtype=np.float32), (128, 1)
        ).astype(ml_dtypes.bfloat16)
        in_maps.append(
            {
                "xh": np.ascontiguousarray(xh_arr),
                "w8": w8.astype(ml_dtypes.bfloat16),
                "cz": czv,
            }
        )
    return in_maps


def kernel(backbone_coords, W, b):
    in_maps = _make_in_maps(backbone_coords, W, b)
    res = _run(in_maps)
    outs = [r["out"].reshape(PB, L, 64) for r in res.results]
    return np.concatenate(outs, axis=0)
